# revision 43
# baseline (speedup 1.0000x reference)
"""Trainium2 Bass kernel for nn_CIN (3-layer CIN / xDeepFM feature-interaction).

Reference computation per layer k (x: (B,39,16), h0 = x):
    z[b,f,g,d] = x[b,f,d] * h[b,g,d]
    cur[b,l,d] = relu(sum_{f,g} z[b,f,g,d] * Wk[f*Fk+g, l] + bk[l])
    h <- cur[:, :64] (layers 0,1);  direct outputs concat'd, summed over d.

Sharding: pure data parallelism, batch 1024 -> 8 cores x 128 rows.

Device layout per core: everything is (partition, n) with n = b*16+d in [0,2048).
The (f,g) interaction pairs are covered by K-chunks of 128 pairs, each either:
  mult path:   DMA: xf = x rows replicated to the chunk's (f,g) rows (SBUF)
               DVE: z = xf * h_rep          (SBUF x SBUF bf16)
  square path: PE:  P = Ssq_c^T @ [x; h]    (x_f - h_g per row, PSUM)
               ACT: z = Square(P)           (x*h = -(1/2)((x-h)^2 - x^2 - h^2);
                                             signs/corrections folded into W)
  both:        PE:  cur += Wc_c^T @ z       (accumulating matmul, PSUM)
Square-path residuals (.5*w*x_f^2 + .5*w*h_g^2) are one extra K-chunk per layer
of host-folded weights against Square([x; h]).

Layer 0 exploits x (x) x symmetry (folds 13 chunks -> 10).

Scheduling notes (learned from traces):
  - The tile framework's cross-engine hazard tracking is tile-granular, so
    every tile is written by exactly one engine (h64: ACT, hrep/rd01/z: DVE)
    and PSUM accumulators are four per-quadrant tiles, making the
    write-after-read on bank reuse quadrant-scoped.
  - PE p-state: dummy-data warm matmuls start right after the NEFF preamble
    and fill every potential gap so the clock never droops to the 1.2GHz
    p-state (which doubles matmul time).
  - Layer boundaries: the next layer's chunk 0 multiplies against the fresh
    64-row h tile directly (two 64-row DVE ops per quadrant), and chunks 1-2
    are square-path chunks whose selector is split into an x-part (available
    immediately) and an h-part, so the PE restarts within ~1us.  The
    replicated h tile (hrep) and the selector rhs (xh) are rebuilt by
    DVE/ACT copies in the shadow of those chunks.
  - GPSIMD measured ~3x slower than its model; used only for head memsets.

All matmul operands are bf16 (fp32 PSUM accumulate).
"""

import numpy as np

B, F, D, L = 1024, 39, 16, 128
NCORES = 8
BC = B // NCORES          # 128 batch rows per core
NF = BC * D               # 2048 free elements per core
HALF = NF // 2            # 1024: psum-bank-pair granule
KP = 128                  # chunk height (partitions)
SPLIT0 = 15               # layer-0: f < SPLIT0 handled by mult path
QS = NF // 4              # 512: psum bank width (quadrant)
SQ_POS12 = (1, 2, 7, 12, 17)   # square-path chunk positions in layers 1/2

_CACHE = {}


def _plan():
    """Structural chunk plan (no weight values), shared by host + device.

    mult: {kind:'mult', fbase, nf}          (l0: f-triples; l1/2: f-pairs)
    sq:   {kind:'sq', slot, pairs: [(urow, vrow, f, g, mode)]}
        urow/vrow: row indices in the bcast rhs tile (l0: xT; l1/2: xh)
        mode 'one' -> W[f,g];  'sym' -> W[f,g] + W[g,f]
        slot: index into the packed selector tensor for this layer group
    """
    layers = []
    # ---- layer 0: 5 sq (folded remainder) + 5 mult (f<15), sq-first ----
    mult = [{"kind": "mult", "fbase": 3 * i, "nf": 3} for i in range(5)]
    entries = []
    for a in range(SPLIT0):
        for b in range(SPLIT0, 39):
            entries.append((b, a, b, a, "one"))       # missing order (f=b, g=a)
    for a in range(SPLIT0, 39):
        for b in range(a + 1, 39):
            entries.append((a, b, a, b, "sym"))
    sq = [
        {"kind": "sq", "slot": si, "pairs": entries[i : i + KP]}
        for si, i in enumerate(range(0, len(entries), KP))
    ]
    order = []
    for i in range(max(len(mult), len(sq))):
        if i < len(sq):
            order.append(sq[i])
        if i < len(mult):
            order.append(mult[i])
    layers.append(order)
    # ---- layers 1, 2: 20 f-pair chunks, SQ_POS12 on the square path ----
    order = []
    nsq = 0
    for i in range(20):
        fb = 2 * i
        nf = 1 if fb == 38 else 2
        if i in SQ_POS12:
            order.append({
                "kind": "sq",
                "slot": nsq,
                "pairs": [
                    (f, 64 + g, f, g, "one")
                    for f in range(fb, min(fb + nf, 39))
                    for g in range(64)
                ],
            })
            nsq += 1
        else:
            order.append({"kind": "mult", "fbase": fb, "nf": nf})
    layers.append(order)
    layers.append(order)
    return layers


PLAN = _plan()
NCH = [len(p) for p in PLAN]
N_MULT0 = sum(1 for c in PLAN[0] if c["kind"] == "mult")
N_MULT12 = sum(1 for c in PLAN[1] if c["kind"] == "mult")
N_SQ0 = sum(1 for c in PLAN[0] if c["kind"] == "sq")
N_SQ12 = sum(1 for c in PLAN[1] if c["kind"] == "sq")

# packed column layout of the cA constant tensor (all bf16, 128 partitions)
CA_WC0 = 0
CA_CORR = CA_WC0 + NCH[0] * L
CA_XT3 = CA_CORR + 3 * L
CA_XF0 = CA_XT3 + NF
CA_COLS = CA_XF0 + N_MULT0 * NF
# packed column layout of the cB constant tensor
CB_SSQ1 = 0
CB_WC1 = CB_SSQ1 + N_SQ12 * KP
CB_WC2 = CB_WC1 + NCH[1] * L
CB_COLS = CB_WC2 + NCH[2] * L
XF_GROUPS = [list(range(g * 4, min((g + 1) * 4, N_MULT12))) for g in range(4)]


def _mult_rows(layer, c):
    """(tile_row, f, g) triples for a mult chunk's 128 z-rows (f>=39 = pad)."""
    fk = 39 if layer == 0 else 64
    out = []
    for p in range(KP):
        f = c["fbase"] + p // fk
        if p // fk >= c["nf"] or f >= 39:
            out.append((p, 39, 0))
        else:
            out.append((p, f, p % fk))
    return out


def _host_consts(W0, W1, W2):
    """Fold reference weights into device constant tensors (fp32, cast later)."""
    Ws = (W0.reshape(39, 39, L), W1.reshape(39, 64, L), W2.reshape(39, 64, L))
    out = {}
    corr_all = np.zeros((128, 3 * L), np.float32)
    for layer in (0, 1, 2):
        W = Ws[layer]
        nch = NCH[layer]
        wc = np.zeros((KP, nch * L), np.float32)
        corr = corr_all[:, layer * L : (layer + 1) * L]
        if layer == 0:
            for a in range(SPLIT0, 39):     # diagonal x_a^2 terms, a >= SPLIT0
                corr[a] += W[a, a]
        if layer < 2:
            ssq_rows = 39 if layer == 0 else 128
            nsq = N_SQ0 if layer == 0 else N_SQ12
            ssq = np.zeros((ssq_rows, nsq * KP), np.float32)
        for ci, c in enumerate(PLAN[layer]):
            if c["kind"] == "mult":
                for p, f, g in _mult_rows(layer, c):
                    if f >= 39:
                        continue
                    wc[p, ci * L : (ci + 1) * L] = W[f, g]
            else:
                si = c["slot"]
                for p, (ur, vr, f, g, mode) in enumerate(c["pairs"]):
                    w = W[f, g] + (W[g, f] if mode == "sym" else 0.0)
                    wc[p, ci * L : (ci + 1) * L] = -0.5 * w
                    corr[ur] += 0.5 * w
                    corr[vr] += 0.5 * w
                    if layer < 2:
                        ssq[ur, si * KP + p] += 1.0
                        ssq[vr, si * KP + p] -= 1.0
        out[f"Wc{layer}"] = wc
        if layer < 2:
            out[f"Ssq{layer}"] = ssq
    out["corr"] = corr_all
    return out


def _build_nc():
    import concourse.bacc as bacc
    import concourse.tile as tile
    from concourse import bass, mybir

    F32 = mybir.dt.float32
    BF16 = mybir.dt.bfloat16
    Relu = mybir.ActivationFunctionType.Relu
    Square = mybir.ActivationFunctionType.Square
    ADD = mybir.AluOpType.add
    MAX = mybir.AluOpType.max
    nc = bacc.Bacc("TRN2", target_bir_lowering=False, debug=False, num_devices=NCORES)

    dram = {}

    def din(name, shape, dt=BF16):
        dram[name] = nc.dram_tensor(name, shape, dt, kind="ExternalInput").ap()

    din("xT", (39, NF))
    din("Ssq0", (39, N_SQ0 * KP))
    din("SsqH", (64, N_SQ12 * KP))
    din("cA", (KP, CA_COLS))
    din("bias", (L, 3), dt=F32)
    din("cB", (KP, CB_COLS))
    for g in range(4):
        din(f"xf{g}", (KP, len(XF_GROUPS[g]) * NF))
    out_d = nc.dram_tensor("out", (256, BC), F32, kind="ExternalOutput").ap()

    with tile.TileContext(nc) as tc:
        with (
            tc.tile_pool(name="const", bufs=1) as cp,
            tc.tile_pool(name="relu", bufs=1) as rp,
            tc.tile_pool(name="zp", bufs=6) as zp,
            tc.tile_pool(name="pbc", bufs=2, space="PSUM") as pbc,
            tc.tile_pool(name="pcur", bufs=1, space="PSUM") as pcur,
        ):
            ct = {}
            for name in dram:
                ct[name] = cp.tile(
                    list(dram[name].shape), dram[name].dtype, tag=name, name=f"c_{name}"
                )
            # engine-exclusive working tiles (one writer engine each)
            dm = cp.tile([39, 640], BF16, tag="dm", name="dm")      # warm dummy
            dmo = cp.tile([39, 128], BF16, tag="dmo", name="dmo")
            xhsq12 = cp.tile([128, NF], BF16, tag="xhsq12", name="xhsq12")  # ACT
            hrep = [None, cp.tile([128, NF], BF16, tag="hrep1", name="hrep1"),
                    cp.tile([128, NF], BF16, tag="hrep2", name="hrep2")]    # ACT
            xh = [None, cp.tile([128, NF], BF16, tag="xh1", name="xh1"),
                  cp.tile([128, NF], BF16, tag="xh2", name="xh2")]          # DMA
            rd01 = rp.tile([128, NF], BF16, tag="rd01", name="rd01")        # DVE
            rdred01 = rp.tile([128, BC], F32, tag="rdred01", name="rdred01")
            rl2 = rp.tile([128, NF], BF16, tag="rl2", name="rl2")           # ACT
            rd2 = rp.tile([128, BC], F32, tag="rd2", name="rd2")            # DVE

            nc.vector.memset(dm[:, :], 0.0)
            nc.scalar.activation(out=dmo[:, :], in_=dm[:, 0:128], func=Square)
            nc.gpsimd.memset(xhsq12[32:64, :], 0.0)
            for nxt in (1, 2):
                nc.gpsimd.memset(xh[nxt][32:64, :], 0.0)

            # DMA issue, consumption order; cA split so early slices land first
            nc.sync.dma_start(out=ct["xT"], in_=dram["xT"])
            nc.sync.dma_start(out=ct["Ssq0"], in_=dram["Ssq0"])
            nc.sync.dma_start(out=ct["SsqH"], in_=dram["SsqH"])
            for cs in (
                slice(CA_WC0, CA_XT3),           # Wc0 + corr
                slice(CA_XT3, CA_XF0),           # xT3
            ):
                nc.sync.dma_start(out=ct["cA"][:, cs], in_=dram["cA"][:, cs])
            for si in range(N_MULT0):
                cs = slice(CA_XF0 + si * NF, CA_XF0 + (si + 1) * NF)
                nc.sync.dma_start(out=ct["cA"][:, cs], in_=dram["cA"][:, cs])
            nc.sync.dma_start(out=ct["bias"], in_=dram["bias"])
            for nxt in (1, 2):
                nc.sync.dma_start(out=xh[nxt][0:39, :], in_=dram["xT"])
            nc.sync.dma_start(
                out=ct["cB"][:, CB_SSQ1:CB_WC2], in_=dram["cB"][:, CB_SSQ1:CB_WC2]
            )
            for g in range(3):
                nc.sync.dma_start(out=ct[f"xf{g}"], in_=dram[f"xf{g}"])
            # xf3 + Wc2 are issued after the first boundary (they are needed
            # late) so the early stream drains before the boundary copies

            def warm(n, tag):
                for wi in range(n):
                    wt = pbc.tile([KP, HALF], F32, tag="bc", name=f"w{tag}{wi}")
                    nc.tensor.matmul(
                        wt[:, 0:512],
                        lhsT=dm[:, 0:128],
                        rhs=dm[:, 128:640],
                        start=True,
                        stop=True,
                    )

            warm(2, "h")

            def wc_view(layer, ci):
                if layer == 0:
                    return ct["cA"][:, CA_WC0 + ci * L : CA_WC0 + (ci + 1) * L]
                base = CB_WC1 if layer == 1 else CB_WC2
                return ct["cB"][:, base + ci * L : base + (ci + 1) * L]

            def ssq_cols(slot):
                return slice(CB_SSQ1 + slot * KP, CB_SSQ1 + (slot + 1) * KP)

            def xf_view(layer, mi):
                if layer == 0:
                    return ct["cA"][:, CA_XF0 + mi * NF : CA_XF0 + (mi + 1) * NF]
                return ct[f"xf{mi // 4}"][:, (mi % 4) * NF : (mi % 4 + 1) * NF]

            def emit_mult(layer, ci, mi, cur, h_rep, start):
                xf = xf_view(layer, mi)
                zt = zp.tile([KP, NF], BF16, tag="z", name=f"zm{layer}_{ci}")
                nc.vector.tensor_mul(zt[:, :], xf[:, :], h_rep[0:KP, :])
                for q in range(4):
                    qs = slice(q * QS, (q + 1) * QS)
                    nc.tensor.matmul(
                        cur[q][:, :],
                        lhsT=wc_view(layer, ci),
                        rhs=zt[:, qs],
                        start=start,
                        stop=False,
                    )

            def sq_phase1(layer, ci, c, xh_t):
                """1-part square chunk, selectors + squares -> zs tiles."""
                sq_rows = 39 if layer == 0 else 128
                ssq = (
                    ct["Ssq0"][0:39, c["slot"] * KP : (c["slot"] + 1) * KP]
                    if layer == 0
                    else ct["cB"][0:128, ssq_cols(c["slot"])]
                )
                zss = []
                for half in range(2):
                    bc = pbc.tile([KP, HALF], F32, tag="bc")
                    for q in range(2):
                        qs = slice(q * 512, (q + 1) * 512)
                        nqs = slice(
                            half * HALF + q * 512, half * HALF + (q + 1) * 512
                        )
                        nc.tensor.matmul(
                            bc[:, qs],
                            lhsT=ssq,
                            rhs=xh_t[0:sq_rows, nqs],
                            start=True,
                            stop=True,
                        )
                    zt = zp.tile([KP, HALF], BF16, tag="zs", name=f"zs{layer}_{ci}")
                    nc.scalar.activation(out=zt[:, :], in_=bc[:, :], func=Square)
                    zss.append(zt)
                return zss

            def sq_phase2(layer, ci, zss, cur, start):
                for half in range(2):
                    for q in range(2):
                        qs = slice(q * 512, (q + 1) * 512)
                        nc.tensor.matmul(
                            cur[2 * half + q][:, :],
                            lhsT=wc_view(layer, ci),
                            rhs=zss[half][:, qs],
                            start=start,
                            stop=False,
                        )

            def sq2p_selx(layer, c):
                """2-part square chunk, x-part selectors (no boundary deps)."""
                ssx = ct["cB"][0:39, ssq_cols(c["slot"])]
                bcs = []
                for half in range(2):
                    bc = pbc.tile([KP, HALF], F32, tag="bc")
                    bcs.append(bc)
                    for q in range(2):
                        qs = slice(q * 512, (q + 1) * 512)
                        nqs = slice(
                            half * HALF + q * 512, half * HALF + (q + 1) * 512
                        )
                        nc.tensor.matmul(
                            bc[:, qs],
                            lhsT=ssx,
                            rhs=ct["xT"][:, nqs],
                            start=True,
                            stop=False,
                        )
                return bcs

            def sq2p_selh(layer, ci, c, bcs, hrep_t):
                """2-part square chunk: h-part selectors + squares -> zs."""
                ssh = ct["SsqH"][0:64, c["slot"] * KP : (c["slot"] + 1) * KP]
                zss = []
                for half in range(2):
                    bc = bcs[half]
                    for q in range(2):
                        qs = slice(q * 512, (q + 1) * 512)
                        nqs = slice(
                            half * HALF + q * 512, half * HALF + (q + 1) * 512
                        )
                        nc.tensor.matmul(
                            bc[:, qs],
                            lhsT=ssh,
                            rhs=hrep_t[0:64, nqs],
                            start=False,
                            stop=True,
                        )
                    zt = zp.tile([KP, HALF], BF16, tag="zs", name=f"zs{layer}_{ci}")
                    nc.scalar.activation(out=zt[:, :], in_=bc[:, :], func=Square)
                    zss.append(zt)
                return zss

            def emit_corr(layer, cur):
                sq_rows = 39 if layer == 0 else 128
                corr = ct["cA"][
                    0:sq_rows, CA_CORR + layer * L : CA_CORR + (layer + 1) * L
                ]
                for q in range(4):
                    qs = slice(q * QS, (q + 1) * QS)
                    nc.tensor.matmul(
                        cur[q][:, :],
                        lhsT=corr,
                        rhs=xhsq12[0:sq_rows, qs],
                        start=False,
                        stop=True,
                    )

            def new_cur(tag):
                return [
                    pcur.tile([128, QS], F32, tag=f"cur{q}", name=f"{tag}_{q}")
                    for q in range(4)
                ]

            def emit_chunks(layer, cur, h_rep_t, xh_t, ci_start, mi_start):
                """Chunk loop with square-path selectors emitted one chunk
                early so their ACT squares hide under the preceding mms."""
                plan = PLAN[layer]
                pending = {}
                mi = mi_start
                for ci in range(ci_start, len(plan)):
                    c = plan[ci]
                    if c["kind"] == "sq":
                        if ci not in pending:
                            pending[ci] = sq_phase1(layer, ci, c, xh_t)
                        sq_phase2(
                            layer, ci, pending.pop(ci), cur,
                            start=(layer == 0 and ci == 0),
                        )
                    else:
                        nx = ci + 1
                        if (
                            nx < len(plan)
                            and plan[nx]["kind"] == "sq"
                            and nx not in pending
                        ):
                            pending[nx] = sq_phase1(layer, nx, plan[nx], xh_t)
                        emit_mult(layer, ci, mi, cur, h_rep_t, start=False)
                        mi += 1
                    if layer == 0:
                        if ci == 3:
                            # x^2 rows of the shared square rhs (all corr use)
                            for q in range(4):
                                qs = slice(q * QS, (q + 1) * QS)
                                nc.scalar.activation(
                                    out=xhsq12[0:39, qs], in_=ct["xT"][:, qs],
                                    func=Square,
                                )
                    elif ci in (5, 10):
                        # fill the DVE-paced dip mid-way through mult runs
                        warm(1, f"l{layer}_{ci}")
                    if layer == 2:
                        # layers 0/1 direct outputs: reduce over d in quadrant
                        # pieces spread across layer 2 (DVE slack)
                        rq = {7: 0, 12: 1, 15: 2, 17: 3}.get(ci)
                        if rq is not None:
                            qs = slice(rq * QS, (rq + 1) * QS)
                            bs = slice(rq * 32, (rq + 1) * 32)
                            nc.vector.tensor_reduce(
                                out=rdred01[:, bs],
                                in_=rd01[:, qs].rearrange("p (b d) -> p b d", d=D),
                                axis=mybir.AxisListType.X,
                                op=mybir.AluOpType.add,
                            )
                            if rq == 3:
                                nc.sync.dma_start(
                                    out=out_d[0:128, :], in_=rdred01
                                )

            # ---------------- layer 0 ----------------
            cur = new_cur("cur0")
            h_rep0 = ct["cA"][:, CA_XT3 : CA_XT3 + NF]  # x tiled to 128 rows
            emit_chunks(0, cur, h_rep0, ct["xT"], 0, 0)
            emit_corr(0, cur)

            # ------------- boundaries + layers 1, 2 -------------
            for layer in (0, 1):
                nxt = layer + 1
                bias_h = ct["bias"][0:64, layer : layer + 1]
                bias_d = ct["bias"][64:128, layer : layer + 1]
                rn, xn = hrep[nxt], xh[nxt]
                cur_n = new_cur(f"cur{nxt}")
                zt0 = zp.tile([KP, NF], BF16, tag="z", name=f"z0_{nxt}")
                xf0 = xf_view(nxt, 0)
                wc0 = wc_view(nxt, 0)
                sq1, sq2 = PLAN[nxt][1], PLAN[nxt][2]
                # x-part selectors of chunk 1: PE work with no boundary deps
                bcs1 = sq2p_selx(nxt, sq1)
                for q in range(4):
                    qs = slice(q * QS, (q + 1) * QS)
                    nc.scalar.activation(
                        out=rn[0:64, qs], in_=cur[q][0:64, :],
                        func=Relu, bias=bias_h, scale=1.0,
                    )
                for q in range(4):
                    qs = slice(q * QS, (q + 1) * QS)
                    nc.vector.tensor_scalar(
                        out=rd01[64 * layer : 64 * layer + 64, qs],
                        in0=cur[q][64:128, :],
                        scalar1=bias_d, scalar2=0.0, op0=ADD, op1=MAX,
                    )
                # chunk 1 h-part + squares hide under the relu/amx window
                zss1 = sq2p_selh(nxt, 1, sq1, bcs1, rn)
                # Replicate h to the upper partitions of hrep.  At the first
                # boundary the input-DMA stream is still draining, so a DMA
                # copy would queue behind it — use ACT relus there; at the
                # second boundary the queues are empty and the DMA is cheaper.
                if layer == 0:
                    for q in range(4):
                        qs = slice(q * QS, (q + 1) * QS)
                        nc.scalar.activation(
                            out=rn[64:128, qs], in_=cur[q][0:64, :],
                            func=Relu, bias=bias_h, scale=1.0,
                        )
                else:
                    nc.gpsimd.dma_start(out=rn[64:128, :], in_=rn[0:64, :])
                # xh h-half for the mid-layer square chunks (first needed ~10us
                # after the boundary, when the input stream has drained)
                nc.gpsimd.dma_start(out=xn[64:128, :], in_=rn[0:64, :])
                for q in range(4):
                    qs = slice(q * QS, (q + 1) * QS)
                    nc.vector.tensor_mul(zt0[:, qs], xf0[:, qs], rn[0:KP, qs])
                    nc.tensor.matmul(
                        cur_n[q][:, :], lhsT=wc0, rhs=zt0[:, qs],
                        start=True, stop=False,
                    )
                sq_phase2(nxt, 1, zss1, cur_n, start=False)
                bcs2 = sq2p_selx(nxt, sq2)
                zss2 = sq2p_selh(nxt, 2, sq2, bcs2, rn)
                sq_phase2(nxt, 2, zss2, cur_n, start=False)
                for hh in range(2):
                    hs = slice(hh * HALF, (hh + 1) * HALF)
                    nc.scalar.activation(
                        out=xhsq12[64:128, hs], in_=rn[0:64, hs], func=Square
                    )

                if layer == 0:
                    nc.sync.dma_start(out=ct["xf3"], in_=dram["xf3"])
                    nc.sync.dma_start(
                        out=ct["cB"][:, CB_WC2:CB_COLS],
                        in_=dram["cB"][:, CB_WC2:CB_COLS],
                    )
                cur = cur_n
                emit_chunks(nxt, cur, rn, xn, 3, 1)
                emit_corr(nxt, cur)

            # ---------------- tail: layer 2 direct ----------------
            warm(10, "tail")  # hold the clock up through the tail relu/reduce
            bias2 = ct["bias"][:, 2:3]
            for q in range(4):
                qs = slice(q * QS, (q + 1) * QS)
                bs = slice(q * 32, (q + 1) * 32)
                nc.scalar.activation(
                    out=rl2[:, qs], in_=cur[q][:, :], func=Relu, bias=bias2,
                    scale=1.0,
                )
                nc.vector.tensor_reduce(
                    out=rd2[:, bs],
                    in_=rl2[:, qs].rearrange("p (b d) -> p b d", d=D),
                    axis=mybir.AxisListType.X,
                    op=mybir.AluOpType.add,
                )
                nc.sync.dma_start(out=out_d[128:256, bs], in_=rd2[:, bs])

    nc.compile()
    return nc


def _get_nc():
    if "nc" not in _CACHE:
        _CACHE["nc"] = _build_nc()
    return _CACHE["nc"]


def _install_profile_shim():
    import sys, types

    if "antenv.axon_hooks" in sys.modules:
        return
    try:
        from trn_agent_boot.trn_boot import _ntff_profile_via_ctypes

        hook = _ntff_profile_via_ctypes("/opt/axon/libaxon_pjrt.so")
    except Exception:
        hook = None
    m = types.ModuleType("antenv.axon_hooks")
    m.get_axon_ntff_profile_hook = lambda: hook
    sys.modules["antenv.axon_hooks"] = m


def _to_bf16(a):
    import ml_dtypes

    return np.ascontiguousarray(a).astype(ml_dtypes.bfloat16)


def host_in_maps(inputs):
    """Host-side sharding + constant folding -> per-core device input maps."""
    x = np.asarray(inputs["x"], np.float32)
    consts = _host_consts(
        np.asarray(inputs["W0"], np.float32),
        np.asarray(inputs["W1"], np.float32),
        np.asarray(inputs["W2"], np.float32),
    )
    bias = np.stack(
        [np.asarray(inputs[f"b{i}"], np.float32) for i in range(3)], axis=1
    )  # (128, 3)

    cA = np.zeros((KP, CA_COLS), np.float32)
    cA[:, CA_WC0:CA_CORR] = consts["Wc0"]
    cA[:, CA_CORR:CA_XT3] = consts["corr"]
    cB = np.zeros((KP, CB_COLS), np.float32)
    cB[0:128, CB_SSQ1:CB_WC1] = consts["Ssq1"]
    cB[:, CB_WC1:CB_WC2] = consts["Wc1"]
    cB[:, CB_WC2:CB_COLS] = consts["Wc2"]

    in_maps = []
    for core in range(NCORES):
        xT = _to_bf16(
            x[core * BC : (core + 1) * BC].transpose(1, 0, 2).reshape(39, NF)
        )
        cAc = cA.copy()
        cAc[:, CA_XT3:CA_XF0] = np.tile(xT, (4, 1))[:KP]
        mi = 0
        for c in PLAN[0]:
            if c["kind"] != "mult":
                continue
            for p, f, g in _mult_rows(0, c):
                cAc[p, CA_XF0 + mi * NF : CA_XF0 + (mi + 1) * NF] = xT[
                    f if f < 39 else 0
                ]
            mi += 1
        xf12 = np.zeros((KP, N_MULT12 * NF), np.float32)
        mi = 0
        for c in PLAN[1]:
            if c["kind"] != "mult":
                continue
            for p, f, g in _mult_rows(1, c):
                xf12[p, mi * NF : (mi + 1) * NF] = xT[f if f < 39 else 0]
            mi += 1
        m = {
            "xT": xT,
            "Ssq0": _to_bf16(consts["Ssq0"]),
            "SsqH": _to_bf16(consts["Ssq1"][64:128, :]),
            "cA": _to_bf16(cAc),
            "bias": np.ascontiguousarray(bias),
            "cB": _to_bf16(cB),
        }
        for g in range(4):
            sl = XF_GROUPS[g]
            m[f"xf{g}"] = _to_bf16(
                xf12[:, sl[0] * NF : (sl[-1] + 1) * NF]
            )
        in_maps.append(m)
    return in_maps


def _enable_ldw_opt():
    """Re-enable walrus's LDWEIGHTS dedupe (consecutive same-weight matmuls)."""
    import concourse.bass_utils as bu

    if getattr(bu, "_ldw_patched", False):
        return
    orig = bu.run_command

    def patched(argv, **kw):
        argv = [
            "--enable-ldw-opt=true" if a == "--enable-ldw-opt=false" else a
            for a in argv
        ]
        return orig(argv, **kw)

    bu.run_command = patched
    bu._ldw_patched = True


def run(inputs, trace=False, trace_cores=None):
    """Run the SPMD kernel; returns (out (1024,256) fp32, BassKernelResults)."""
    from concourse.bass_utils import run_bass_kernel_spmd

    _install_profile_shim()
    in_maps = host_in_maps(inputs)
    nc = _get_nc()
    res = run_bass_kernel_spmd(
        nc, in_maps, list(range(NCORES)), trace=trace, trace_cores=trace_cores
    )
    out = np.concatenate(
        [res.results[c]["out"].T for c in range(NCORES)], axis=0
    ).astype(np.float32)
    return out, res


def kernel(**inputs):
    out, _ = run(inputs, trace=False)
    return out
